# revision 4
# baseline (speedup 1.0000x reference)
"""GNN message-passing (GAT-style) Trainium2 kernel.

out = (N * A) @ (x @ W) with A_ij = LeakyReLU([m_i||m_j] a) row-normalized
over the edge sparsity pattern.

Strategy (8 NeuronCores, SPMD, no collectives):
  k1: core c owns a 6272-node src slice; one fused f32 matmul per 128-node
      tile with rhs = [W | W@a_src | W@a_dst] yields m (cast to fp16) and
      the per-node scalars s = m@a_src, t = m@a_dst.
  host (indexing only, no float math): partitions edges by src slice,
      degree-sorts nodes into 49 tiles of 128, builds a k-major slot layout
      (K_i slots per tile, slot k of tile i = one edge per src node), and
      gathers the fp16 m rows of every edge's dst into one contiguous
      G tensor per core, plus slot-aligned t[dst], n_vals, s arrays.
  k2: per core: e = LeakyReLU(s+t) and w = e*n_vals in a few large vector
      ops; per tile ONE broadcast multiply builds all K diag(w_k) blocks
      from a tiled-identity constant; K PSUM-accumulated matmuls
      U = sum_k diag(w_k) @ G_k do the scatter; rs = row-sums of e; the
      PSUM->SBUF eviction applies 1/rs via the scalar engine. Final out is
      written directly (no third kernel); host just un-permutes rows.

All floating-point math runs on device; the host only shards, sorts, pads,
gathers and re-indexes.
"""

import os
import sys

for _p in ("/opt/trn_rl_repo", "/root/.axon_site/_ro/trn_rl_repo"):
    if os.path.isdir(_p) and _p not in sys.path:
        sys.path.insert(0, _p)
        break

os.environ.setdefault("MYCRO_LOCAL_CACHE", "1")

# The agent image's antenv package lacks axon_hooks; bass_utils imports it
# when tracing is requested. Install a functional shim so trace runs work
# (and degrade to no-trace when the axon .so is unavailable).
try:
    import antenv.axon_hooks  # noqa: F401
except ImportError:
    import types

    import antenv

    _hooks = types.ModuleType("antenv.axon_hooks")
    _HOOK = [None]
    _hooks.set_axon_ntff_profile_hook = lambda h: _HOOK.__setitem__(0, h)
    _hooks.get_axon_ntff_profile_hook = lambda: _HOOK[0]
    sys.modules["antenv.axon_hooks"] = _hooks
    antenv.axon_hooks = _hooks
    try:
        from trn_agent_boot.trn_boot import _ntff_profile_via_ctypes

        if os.path.exists("/opt/axon/libaxon_pjrt.so"):
            _hooks.set_axon_ntff_profile_hook(
                _ntff_profile_via_ctypes("/opt/axon/libaxon_pjrt.so")
            )
    except Exception:
        pass

import numpy as np

import concourse.bacc as bacc
import concourse.bass as bass
import concourse.mybir as mybir
import concourse.tile as tile
from concourse.bass_utils import run_bass_kernel_spmd

F32 = mybir.dt.float32
F16 = mybir.dt.float16

P = 128
N_CELLS = 50000
D_IN = 256
D_OUT = 128
NEG_SLOPE = 0.2

N_CORES = 8
NPAD = 50176          # 8 * 6272
CHUNK = 6272          # nodes per core (49 tiles)
NT1 = CHUNK // P      # 49

TRACE = bool(int(os.environ.get("GNN_TRACE", "0")))

last_exec_times = []


def _run(nc, in_maps):
    nc.compile()
    res = run_bass_kernel_spmd(
        nc, in_maps, core_ids=list(range(N_CORES)), trace=TRACE,
    )
    if res.exec_time_ns is not None:
        last_exec_times.append(res.exec_time_ns)
    return res.results


# ------------------------------------------------------------------ k1 ----
def _build_k1():
    nc = bacc.Bacc("TRN2", target_bir_lowering=False, debug=False,
                   enable_asserts=False, num_devices=N_CORES)
    xT_d = nc.dram_tensor("xT", [D_IN, CHUNK], F32, kind="ExternalInput").ap()
    W_d = nc.dram_tensor("Wm", [D_IN, D_OUT], F32, kind="ExternalInput").ap()
    WT_d = nc.dram_tensor("WT", [D_OUT, D_IN], F32, kind="ExternalInput").ap()
    a2_d = nc.dram_tensor("a2", [P, 2], F32, kind="ExternalInput").ap()
    m16_d = nc.dram_tensor("m16", [CHUNK, D_OUT], F16, kind="ExternalOutput").ap()
    st_d = nc.dram_tensor("st", [P, 2 * NT1], F32, kind="ExternalOutput").ap()

    with tile.TileContext(nc) as tc:
        with (
            tc.tile_pool(name="const", bufs=1) as cp,
            tc.tile_pool(name="xp", bufs=4) as xp,
            tc.tile_pool(name="work", bufs=4) as wp,
            tc.tile_pool(name="psum", bufs=4, space="PSUM") as pp,
        ):
            rhs0 = cp.tile([P, D_OUT + 2], F32, tag="rhs0")
            rhs1 = cp.tile([P, D_OUT + 2], F32, tag="rhs1")
            nc.sync.dma_start(out=rhs0[:, 0:D_OUT], in_=W_d[0:P, :])
            nc.sync.dma_start(out=rhs1[:, 0:D_OUT], in_=W_d[P:D_IN, :])
            WT_sb = cp.tile([P, D_IN], F32, tag="WT")
            nc.sync.dma_start(out=WT_sb[:], in_=WT_d[:])
            a2_sb = cp.tile([P, 2], F32, tag="a2")
            nc.sync.dma_start(out=a2_sb[:], in_=a2_d[:])

            # [W @ a_src | W @ a_dst] rows 0:128 and 128:256
            wa0 = pp.tile([P, 2], F32, tag="wa")
            nc.tensor.matmul(out=wa0[:], lhsT=WT_sb[:, 0:P], rhs=a2_sb[:],
                             start=True, stop=True)
            nc.vector.tensor_copy(out=rhs0[:, D_OUT:D_OUT + 2], in_=wa0[:])
            wa1 = pp.tile([P, 2], F32, tag="wa")
            nc.tensor.matmul(out=wa1[:], lhsT=WT_sb[:, P:D_IN], rhs=a2_sb[:],
                             start=True, stop=True)
            nc.vector.tensor_copy(out=rhs1[:, D_OUT:D_OUT + 2], in_=wa1[:])

            st_acc = cp.tile([P, 2 * NT1], F32, tag="stacc")
            TPC = 7                       # tiles per chunk; 49 = 7 * 7
            for ch in range(NT1 // TPC):
                c0 = ch * TPC * P
                xc0 = xp.tile([P, TPC * P], F32, tag="xc0")
                xc1 = xp.tile([P, TPC * P], F32, tag="xc1")
                nc.sync.dma_start(out=xc0[:], in_=xT_d[0:P, c0:c0 + TPC * P])
                nc.sync.dma_start(out=xc1[:], in_=xT_d[P:D_IN, c0:c0 + TPC * P])
                for j in range(TPC):
                    i = ch * TPC + j
                    mp = pp.tile([P, D_OUT + 2], F32, tag="mp")
                    nc.tensor.matmul(out=mp[:], lhsT=xc0[:, bass.ts(j, P)],
                                     rhs=rhs0[:], start=True, stop=False)
                    nc.tensor.matmul(out=mp[:], lhsT=xc1[:, bass.ts(j, P)],
                                     rhs=rhs1[:], start=False, stop=True)
                    m16t = wp.tile([P, D_OUT], F16, tag="m16t")
                    nc.vector.tensor_copy(out=m16t[:], in_=mp[:, 0:D_OUT])
                    nc.scalar.copy(out=st_acc[:, 2 * i:2 * i + 2],
                                   in_=mp[:, D_OUT:D_OUT + 2])
                    nc.sync.dma_start(out=m16_d[bass.ts(i, P), :], in_=m16t[:])
            nc.sync.dma_start(out=st_d[:], in_=st_acc[:])
    return nc


# ------------------------------------------------------------------ k2 ----
def _build_k2(K_sched):
    K_sched = [int(k) for k in K_sched]
    KTOT = sum(K_sched)
    KMAX = max(K_sched)
    nt = len(K_sched)
    nc = bacc.Bacc("TRN2", target_bir_lowering=False, debug=False,
                   enable_asserts=False, num_devices=N_CORES)
    G_d = nc.dram_tensor("G", [P, KTOT * D_OUT], F16, kind="ExternalInput").ap()
    t_d = nc.dram_tensor("t_s", [P, KTOT], F32, kind="ExternalInput").ap()
    nv_d = nc.dram_tensor("nv_s", [P, KTOT], F32, kind="ExternalInput").ap()
    s_d = nc.dram_tensor("s_s", [P, KTOT], F32, kind="ExternalInput").ap()
    idt_d = nc.dram_tensor("idt", [P, KMAX * D_OUT], F16, kind="ExternalInput").ap()
    o_d = nc.dram_tensor("outc", [CHUNK, D_OUT], F32, kind="ExternalOutput").ap()

    with tile.TileContext(nc) as tc:
        with (
            tc.tile_pool(name="const", bufs=1) as cp,
            tc.tile_pool(name="gpool", bufs=3) as gp,
            tc.tile_pool(name="dpool", bufs=3) as dp,
            tc.tile_pool(name="opool", bufs=4) as op,
            tc.tile_pool(name="psum", bufs=4, space="PSUM") as pp,
        ):
            idt = cp.tile([P, KMAX * D_OUT], F16, tag="idt")
            nc.sync.dma_start(out=idt[:], in_=idt_d[:])
            t_sb = cp.tile([P, KTOT], F32, tag="t")
            nv_sb = cp.tile([P, KTOT], F32, tag="nv")
            s_sb = cp.tile([P, KTOT], F32, tag="s")
            nc.sync.dma_start(out=t_sb[:], in_=t_d[:])
            nc.sync.dma_start(out=nv_sb[:], in_=nv_d[:])
            nc.sync.dma_start(out=s_sb[:], in_=s_d[:])

            z_sb = cp.tile([P, KTOT], F32, tag="z")
            zp_sb = cp.tile([P, KTOT], F32, tag="zp")
            e_sb = cp.tile([P, KTOT], F32, tag="e")
            w16 = cp.tile([P, KTOT], F16, tag="w16")
            rs = cp.tile([P, nt], F32, tag="rs")
            rinv = cp.tile([P, nt], F32, tag="rinv")

            # e = LeakyReLU(t + s) = max(z,0) + NEG_SLOPE*min(z,0)
            nc.vector.tensor_add(out=z_sb[:], in0=t_sb[:], in1=s_sb[:])
            nc.vector.tensor_scalar(
                out=zp_sb[:], in0=z_sb[:], scalar1=0.0, scalar2=None,
                op0=mybir.AluOpType.max)
            nc.vector.tensor_scalar(
                out=z_sb[:], in0=z_sb[:], scalar1=0.0, scalar2=NEG_SLOPE,
                op0=mybir.AluOpType.min, op1=mybir.AluOpType.mult)
            nc.vector.tensor_add(out=e_sb[:], in0=zp_sb[:], in1=z_sb[:])
            # w = e * n_vals (fp16 for the diag blocks)
            nc.vector.tensor_mul(out=w16[:], in0=e_sb[:], in1=nv_sb[:])
            # rs_i = per-tile row sums of e; rinv = 1/rs
            off = 0
            for i in range(nt):
                K = K_sched[i]
                nc.vector.reduce_sum(out=rs[:, i:i + 1],
                                     in_=e_sb[:, off:off + K],
                                     axis=mybir.AxisListType.X)
                off += K
            nc.vector.reciprocal(out=rinv[:], in_=rs[:])

            off = 0
            for i in range(nt):
                K = K_sched[i]
                G_t = gp.tile([P, KMAX * D_OUT], F16, tag="G")
                nc.sync.dma_start(
                    out=G_t[:, 0:K * D_OUT],
                    in_=G_d[:, off * D_OUT:(off + K) * D_OUT])
                diag = dp.tile([P, KMAX * D_OUT], F16, tag="diag")
                w_b = (w16[:, off:off + K]
                       .unsqueeze(2).broadcast_to([P, K, D_OUT]))
                eng = nc.vector if i % 2 == 0 else nc.gpsimd
                eng.tensor_mul(
                    out=diag[:, 0:K * D_OUT].rearrange(
                        "p (k q) -> p k q", q=D_OUT),
                    in0=idt[:, 0:K * D_OUT].rearrange(
                        "p (k q) -> p k q", q=D_OUT),
                    in1=w_b)
                U_ps = pp.tile([P, D_OUT], F32, tag="U")
                for k in range(K):
                    nc.tensor.matmul(
                        out=U_ps[:],
                        lhsT=diag[:, k * D_OUT:(k + 1) * D_OUT],
                        rhs=G_t[:, k * D_OUT:(k + 1) * D_OUT],
                        start=(k == 0), stop=(k == K - 1))
                U_sb = op.tile([P, D_OUT], F32, tag="Usb")
                nc.scalar.mul(out=U_sb[:], in_=U_ps[:], mul=rinv[:, i:i + 1])
                nc.sync.dma_start(out=o_d[bass.ts(i, P), :], in_=U_sb[:])
                off += K
    return nc


# ------------------------------------------------------------ host prep ----
def _prep_k2(n_src, n_dst, n_vals, m_all, s_all, t_all):
    """Slot layouts per core. Pure indexing / permutation, no float math."""
    cores = []
    for c in range(N_CORES):
        sel = (n_src // CHUNK) == c
        es = n_src[sel] - c * CHUNK
        ed = n_dst[sel]
        ev = n_vals[sel]
        deg = np.bincount(es, minlength=CHUNK)
        order = np.argsort(-deg, kind="stable")       # rank -> rel id
        rank = np.empty(CHUNK, dtype=np.int64)
        rank[order] = np.arange(CHUNK)
        eo = np.argsort(rank[es], kind="stable")
        ed = ed[eo]
        ev = ev[eo]
        deg_sorted = deg[order]
        offs = np.zeros(CHUNK + 1, dtype=np.int64)
        np.cumsum(deg_sorted, out=offs[1:])
        Ks = np.maximum(
            deg_sorted.reshape(NT1, P).max(axis=1), 1).astype(np.int64)
        cores.append(dict(c=c, ed=ed, ev=ev, offs=offs, order=order,
                          rank=rank, Ks=Ks, deg_sorted=deg_sorted))
    K_sched = np.stack([c["Ks"] for c in cores]).max(axis=0)
    KTOT = int(K_sched.sum())
    KMAX = int(K_sched.max())

    for c in cores:
        cc = c["c"]
        dst_all = np.zeros((P, KTOT), dtype=np.int64)
        t_s = np.zeros((P, KTOT), dtype=np.float32)
        nv_s = np.zeros((P, KTOT), dtype=np.float32)
        s_s = np.zeros((P, KTOT), dtype=np.float32)
        off = 0
        for i in range(NT1):
            K = int(K_sched[i])
            nodes = c["order"][i * P:(i + 1) * P]        # rel ids
            s_node = s_all[cc * CHUNK + nodes]           # [128]
            d = c["deg_sorted"][i * P:(i + 1) * P]       # [128]
            base = c["offs"][i * P:(i + 1) * P]          # [128]
            sl = np.arange(K)[None, :]
            valid = sl < d[:, None]                      # [128, K]
            eidx = np.minimum(base[:, None] + sl, max(len(c["ed"]) - 1, 0))
            dst_all[:, off:off + K] = np.where(valid, c["ed"][eidx], 0)
            nv_s[:, off:off + K] = np.where(valid, c["ev"][eidx], 0.0)
            t_s[:, off:off + K] = np.where(
                valid, t_all[dst_all[:, off:off + K]], -s_node[:, None])
            s_s[:, off:off + K] = s_node[:, None]
            off += K
        G = m_all[dst_all]                               # [128, KTOT, 128] f16
        c["G"] = np.ascontiguousarray(G.reshape(P, KTOT * D_OUT))
        c["t_s"] = t_s
        c["nv_s"] = nv_s
        c["s_s"] = s_s
    idt = np.ascontiguousarray(
        np.tile(np.eye(P, dtype=np.float16), (1, KMAX)))
    return cores, K_sched, idt


# ---------------------------------------------------------------- main ----
def kernel(x, W, a, n_vals, n_src, n_dst):
    global last_exec_times
    last_exec_times = []
    x = np.ascontiguousarray(np.asarray(x, dtype=np.float32))
    W = np.ascontiguousarray(np.asarray(W, dtype=np.float32))
    a = np.ascontiguousarray(np.asarray(a, dtype=np.float32))
    n_vals = np.ascontiguousarray(np.asarray(n_vals, dtype=np.float32))
    n_src = np.asarray(n_src, dtype=np.int64)
    n_dst = np.asarray(n_dst, dtype=np.int64)

    # ---- k1
    x_pad = np.zeros((NPAD, D_IN), dtype=np.float32)
    x_pad[:N_CELLS] = x
    a2 = np.ascontiguousarray(
        np.stack([a[:D_OUT, 0], a[D_OUT:, 0]], axis=1))  # [128, 2]
    WT = np.ascontiguousarray(W.T)
    in1 = [
        dict(xT=np.ascontiguousarray(x_pad[c * CHUNK:(c + 1) * CHUNK].T),
             Wm=W, WT=WT, a2=a2)
        for c in range(N_CORES)
    ]
    r1 = _run(_build_k1(), in1)
    m_all = np.concatenate([r1[c]["m16"] for c in range(N_CORES)], axis=0)
    s_all = np.concatenate(
        [r1[c]["st"][:, 0::2].T.reshape(-1) for c in range(N_CORES)])
    t_all = np.concatenate(
        [r1[c]["st"][:, 1::2].T.reshape(-1) for c in range(N_CORES)])

    # ---- k2
    cores, K_sched, idt = _prep_k2(n_src, n_dst, n_vals, m_all, s_all, t_all)
    in2 = [
        dict(G=c["G"], t_s=c["t_s"], nv_s=c["nv_s"], s_s=c["s_s"], idt=idt)
        for c in cores
    ]
    r2 = _run(_build_k2(K_sched), in2)

    out = np.empty((NPAD, D_OUT), dtype=np.float32)
    for c in cores:
        cc = c["c"]
        out[cc * CHUNK:(cc + 1) * CHUNK] = r2[cc]["outc"][c["rank"]]
    return np.ascontiguousarray(out[:N_CELLS])


# revision 7
# speedup vs baseline: 1.0602x; 1.0602x over previous
"""GNN message-passing (GAT-style) Trainium2 kernel.

out = (N * A) @ (x @ W) with A_ij = LeakyReLU([m_i||m_j] a) row-normalized
over the edge sparsity pattern.

Strategy (8 NeuronCores, SPMD, no collectives):
  k1: core c owns a 6272-node src slice; one fused f32 matmul per 128-node
      tile with rhs = [W | W@a_src | W@a_dst] yields m (cast to fp16) and
      the per-node scalars s = m@a_src, t = m@a_dst.
  host (indexing only, no float math): partitions edges by src slice,
      degree-sorts nodes into 49 tiles of 128, builds a k-major slot layout
      (K_i slots per tile, slot k of tile i = one edge per src node), and
      gathers the fp16 m rows of every edge's dst into one contiguous
      G tensor per core, plus slot-aligned t[dst], n_vals, s arrays.
  k2: per core: e = LeakyReLU(s+t) and w = e*n_vals in a few large vector
      ops; per tile ONE broadcast multiply builds all K diag(w_k) blocks
      from a tiled-identity constant; K PSUM-accumulated matmuls
      U = sum_k diag(w_k) @ G_k do the scatter; rs = row-sums of e; the
      PSUM->SBUF eviction applies 1/rs via the scalar engine. Final out is
      written directly (no third kernel); host just un-permutes rows.

All floating-point math runs on device; the host only shards, sorts, pads,
gathers and re-indexes.
"""

import os
import sys

for _p in ("/opt/trn_rl_repo", "/root/.axon_site/_ro/trn_rl_repo"):
    if os.path.isdir(_p) and _p not in sys.path:
        sys.path.insert(0, _p)
        break

os.environ.setdefault("MYCRO_LOCAL_CACHE", "1")

# The agent image's antenv package lacks axon_hooks; bass_utils imports it
# when tracing is requested. Install a functional shim so trace runs work
# (and degrade to no-trace when the axon .so is unavailable).
try:
    import antenv.axon_hooks  # noqa: F401
except ImportError:
    import types

    import antenv

    _hooks = types.ModuleType("antenv.axon_hooks")
    _HOOK = [None]
    _hooks.set_axon_ntff_profile_hook = lambda h: _HOOK.__setitem__(0, h)
    _hooks.get_axon_ntff_profile_hook = lambda: _HOOK[0]
    sys.modules["antenv.axon_hooks"] = _hooks
    antenv.axon_hooks = _hooks
    try:
        from trn_agent_boot.trn_boot import _ntff_profile_via_ctypes

        if os.path.exists("/opt/axon/libaxon_pjrt.so"):
            _hooks.set_axon_ntff_profile_hook(
                _ntff_profile_via_ctypes("/opt/axon/libaxon_pjrt.so")
            )
    except Exception:
        pass

import numpy as np

import concourse.bacc as bacc
import concourse.bass as bass
import concourse.mybir as mybir
import concourse.tile as tile
from concourse.bass_utils import run_bass_kernel_spmd

F32 = mybir.dt.float32
F16 = mybir.dt.float16

P = 128
N_CELLS = 50000
D_IN = 256
D_OUT = 128
NEG_SLOPE = 0.2

N_CORES = 8
NPAD = 50176          # 8 * 6272
CHUNK = 6272          # nodes per core (49 tiles)
NT1 = CHUNK // P      # 49

TRACE = bool(int(os.environ.get("GNN_TRACE", "0")))

last_exec_times = []


def _run(nc, in_maps):
    nc.compile()
    res = run_bass_kernel_spmd(
        nc, in_maps, core_ids=list(range(N_CORES)), trace=TRACE,
    )
    if res.exec_time_ns is not None:
        last_exec_times.append(res.exec_time_ns)
    return res.results


# ------------------------------------------------------------------ k1 ----
def _build_k1():
    nc = bacc.Bacc("TRN2", target_bir_lowering=False, debug=False,
                   enable_asserts=False, num_devices=N_CORES)
    xT_d = nc.dram_tensor("xT", [D_IN, CHUNK], F32, kind="ExternalInput").ap()
    W_d = nc.dram_tensor("Wm", [D_IN, D_OUT], F32, kind="ExternalInput").ap()
    WT_d = nc.dram_tensor("WT", [D_OUT, D_IN], F32, kind="ExternalInput").ap()
    a2_d = nc.dram_tensor("a2", [P, 2], F32, kind="ExternalInput").ap()
    m16_d = nc.dram_tensor("m16", [CHUNK, D_OUT], F16, kind="ExternalOutput").ap()
    st_d = nc.dram_tensor("st", [P, 2 * NT1], F32, kind="ExternalOutput").ap()

    with tile.TileContext(nc) as tc:
        with (
            tc.tile_pool(name="const", bufs=1) as cp,
            tc.tile_pool(name="xp", bufs=4) as xp,
            tc.tile_pool(name="work", bufs=4) as wp,
            tc.tile_pool(name="psum", bufs=4, space="PSUM") as pp,
        ):
            rhs0 = cp.tile([P, D_OUT + 2], F32, tag="rhs0")
            rhs1 = cp.tile([P, D_OUT + 2], F32, tag="rhs1")
            nc.sync.dma_start(out=rhs0[:, 0:D_OUT], in_=W_d[0:P, :])
            nc.sync.dma_start(out=rhs1[:, 0:D_OUT], in_=W_d[P:D_IN, :])
            WT_sb = cp.tile([P, D_IN], F32, tag="WT")
            nc.sync.dma_start(out=WT_sb[:], in_=WT_d[:])
            a2_sb = cp.tile([P, 2], F32, tag="a2")
            nc.sync.dma_start(out=a2_sb[:], in_=a2_d[:])

            # [W @ a_src | W @ a_dst] rows 0:128 and 128:256
            wa0 = pp.tile([P, 2], F32, tag="wa")
            nc.tensor.matmul(out=wa0[:], lhsT=WT_sb[:, 0:P], rhs=a2_sb[:],
                             start=True, stop=True)
            nc.vector.tensor_copy(out=rhs0[:, D_OUT:D_OUT + 2], in_=wa0[:])
            wa1 = pp.tile([P, 2], F32, tag="wa")
            nc.tensor.matmul(out=wa1[:], lhsT=WT_sb[:, P:D_IN], rhs=a2_sb[:],
                             start=True, stop=True)
            nc.vector.tensor_copy(out=rhs1[:, D_OUT:D_OUT + 2], in_=wa1[:])

            st_acc = cp.tile([P, 2 * NT1], F32, tag="stacc")
            TPC = 7                       # tiles per chunk; 49 = 7 * 7
            for ch in range(NT1 // TPC):
                c0 = ch * TPC * P
                xc0 = xp.tile([P, TPC * P], F32, tag="xc0")
                xc1 = xp.tile([P, TPC * P], F32, tag="xc1")
                nc.sync.dma_start(out=xc0[:], in_=xT_d[0:P, c0:c0 + TPC * P])
                nc.sync.dma_start(out=xc1[:], in_=xT_d[P:D_IN, c0:c0 + TPC * P])
                for j in range(TPC):
                    i = ch * TPC + j
                    mp = pp.tile([P, D_OUT + 2], F32, tag="mp")
                    nc.tensor.matmul(out=mp[:], lhsT=xc0[:, bass.ts(j, P)],
                                     rhs=rhs0[:], start=True, stop=False)
                    nc.tensor.matmul(out=mp[:], lhsT=xc1[:, bass.ts(j, P)],
                                     rhs=rhs1[:], start=False, stop=True)
                    m16t = wp.tile([P, D_OUT], F16, tag="m16t")
                    nc.vector.tensor_copy(out=m16t[:], in_=mp[:, 0:D_OUT])
                    nc.scalar.copy(out=st_acc[:, 2 * i:2 * i + 2],
                                   in_=mp[:, D_OUT:D_OUT + 2])
                    nc.sync.dma_start(out=m16_d[bass.ts(i, P), :], in_=m16t[:])
            nc.sync.dma_start(out=st_d[:], in_=st_acc[:])
    return nc


# ------------------------------------------------------------------ k2 ----
def _build_k2(K_sched):
    K_sched = [int(k) for k in K_sched]
    KTOT = sum(K_sched)
    KMAX = max(K_sched)
    nt = len(K_sched)
    nc = bacc.Bacc("TRN2", target_bir_lowering=False, debug=False,
                   enable_asserts=False, num_devices=N_CORES)
    G_d = nc.dram_tensor("G", [P, KTOT * D_OUT], F16, kind="ExternalInput").ap()
    t_d = nc.dram_tensor("t_s", [P, KTOT], F32, kind="ExternalInput").ap()
    nv_d = nc.dram_tensor("nv_s", [P, KTOT], F32, kind="ExternalInput").ap()
    s_d = nc.dram_tensor("s_s", [P, KTOT], F32, kind="ExternalInput").ap()
    idt_d = nc.dram_tensor("idt", [P, KMAX * D_OUT], F16, kind="ExternalInput").ap()
    o_d = nc.dram_tensor("outc", [CHUNK, D_OUT], F32, kind="ExternalOutput").ap()

    with tile.TileContext(nc) as tc:
        with (
            tc.tile_pool(name="const", bufs=1) as cp,
            tc.tile_pool(name="gpool", bufs=3) as gp,
            tc.tile_pool(name="dpoolv", bufs=3) as dpv,
            tc.tile_pool(name="dpoolg", bufs=3) as dpg,
            tc.tile_pool(name="opool", bufs=4) as op,
            tc.tile_pool(name="psum", bufs=4, space="PSUM") as pp,
        ):
            idt = cp.tile([P, KMAX * D_OUT], F16, tag="idt")
            nc.sync.dma_start(out=idt[:], in_=idt_d[:])
            t_sb = cp.tile([P, KTOT], F32, tag="t")
            nv_sb = cp.tile([P, KTOT], F32, tag="nv")
            s_sb = cp.tile([P, KTOT], F32, tag="s")
            nc.sync.dma_start(out=t_sb[:], in_=t_d[:])
            nc.sync.dma_start(out=nv_sb[:], in_=nv_d[:])
            nc.sync.dma_start(out=s_sb[:], in_=s_d[:])

            z_sb = cp.tile([P, KTOT], F32, tag="z")
            zp_sb = cp.tile([P, KTOT], F32, tag="zp")
            e_sb = cp.tile([P, KTOT], F32, tag="e")
            w16 = cp.tile([P, KTOT], F16, tag="w16")
            rs = cp.tile([P, nt], F32, tag="rs")
            rinv = cp.tile([P, nt], F32, tag="rinv")

            # e = LeakyReLU(t + s) = max(z,0) + NEG_SLOPE*min(z,0)
            nc.vector.tensor_add(out=z_sb[:], in0=t_sb[:], in1=s_sb[:])
            nc.vector.tensor_scalar(
                out=zp_sb[:], in0=z_sb[:], scalar1=0.0, scalar2=None,
                op0=mybir.AluOpType.max)
            nc.vector.tensor_scalar(
                out=z_sb[:], in0=z_sb[:], scalar1=0.0, scalar2=NEG_SLOPE,
                op0=mybir.AluOpType.min, op1=mybir.AluOpType.mult)
            nc.vector.tensor_add(out=e_sb[:], in0=zp_sb[:], in1=z_sb[:])
            # w = e * n_vals (fp16 for the diag blocks)
            nc.vector.tensor_mul(out=w16[:], in0=e_sb[:], in1=nv_sb[:])
            # rs_i = per-tile row sums of e; rinv = 1/rs
            off = 0
            for i in range(nt):
                K = K_sched[i]
                nc.vector.reduce_sum(out=rs[:, i:i + 1],
                                     in_=e_sb[:, off:off + K],
                                     axis=mybir.AxisListType.X)
                off += K
            nc.vector.reciprocal(out=rinv[:], in_=rs[:])

            off = 0
            for i in range(nt):
                K = K_sched[i]
                G_t = gp.tile([P, KMAX * D_OUT], F16, tag="G")
                nc.sync.dma_start(
                    out=G_t[:, 0:K * D_OUT],
                    in_=G_d[:, off * D_OUT:(off + K) * D_OUT])
                on_v = (i % 3) < 2   # vector gets 2/3 of tiles
                dp_ = dpv if on_v else dpg
                eng = nc.vector if on_v else nc.gpsimd
                diag = dp_.tile([P, KMAX * D_OUT], F16, tag="diag")
                w_b = (w16[:, off:off + K]
                       .unsqueeze(2).broadcast_to([P, K, D_OUT]))
                eng.tensor_mul(
                    out=diag[:, 0:K * D_OUT].rearrange(
                        "p (k q) -> p k q", q=D_OUT),
                    in0=idt[:, 0:K * D_OUT].rearrange(
                        "p (k q) -> p k q", q=D_OUT),
                    in1=w_b)
                U_ps = pp.tile([P, D_OUT], F32, tag="U")
                for k in range(K):
                    nc.tensor.matmul(
                        out=U_ps[:],
                        lhsT=diag[:, k * D_OUT:(k + 1) * D_OUT],
                        rhs=G_t[:, k * D_OUT:(k + 1) * D_OUT],
                        start=(k == 0), stop=(k == K - 1))
                U_sb = op.tile([P, D_OUT], F32, tag="Usb")
                nc.scalar.mul(out=U_sb[:], in_=U_ps[:], mul=rinv[:, i:i + 1])
                nc.sync.dma_start(out=o_d[bass.ts(i, P), :], in_=U_sb[:])
                off += K
    return nc


# ------------------------------------------------------------ host prep ----
def _prep_k2(n_src, n_dst, n_vals, m_all, s_all, t_all):
    """Slot layouts per core. Pure indexing / permutation, no float math."""
    cores = []
    for c in range(N_CORES):
        sel = (n_src // CHUNK) == c
        es = n_src[sel] - c * CHUNK
        ed = n_dst[sel]
        ev = n_vals[sel]
        deg = np.bincount(es, minlength=CHUNK)
        order = np.argsort(-deg, kind="stable")       # rank -> rel id
        rank = np.empty(CHUNK, dtype=np.int64)
        rank[order] = np.arange(CHUNK)
        eo = np.argsort(rank[es], kind="stable")
        ed = ed[eo]
        ev = ev[eo]
        deg_sorted = deg[order]
        offs = np.zeros(CHUNK + 1, dtype=np.int64)
        np.cumsum(deg_sorted, out=offs[1:])
        Ks = np.maximum(
            deg_sorted.reshape(NT1, P).max(axis=1), 1).astype(np.int64)
        cores.append(dict(c=c, ed=ed, ev=ev, offs=offs, order=order,
                          rank=rank, Ks=Ks, deg_sorted=deg_sorted))
    K_sched = np.stack([c["Ks"] for c in cores]).max(axis=0)
    KTOT = int(K_sched.sum())
    KMAX = int(K_sched.max())

    for c in cores:
        cc = c["c"]
        dst_all = np.zeros((P, KTOT), dtype=np.int64)
        t_s = np.zeros((P, KTOT), dtype=np.float32)
        nv_s = np.zeros((P, KTOT), dtype=np.float32)
        s_s = np.zeros((P, KTOT), dtype=np.float32)
        off = 0
        for i in range(NT1):
            K = int(K_sched[i])
            nodes = c["order"][i * P:(i + 1) * P]        # rel ids
            s_node = s_all[cc * CHUNK + nodes]           # [128]
            d = c["deg_sorted"][i * P:(i + 1) * P]       # [128]
            base = c["offs"][i * P:(i + 1) * P]          # [128]
            sl = np.arange(K)[None, :]
            valid = sl < d[:, None]                      # [128, K]
            eidx = np.minimum(base[:, None] + sl, max(len(c["ed"]) - 1, 0))
            dst_all[:, off:off + K] = np.where(valid, c["ed"][eidx], 0)
            nv_s[:, off:off + K] = np.where(valid, c["ev"][eidx], 0.0)
            t_s[:, off:off + K] = np.where(
                valid, t_all[dst_all[:, off:off + K]], -s_node[:, None])
            s_s[:, off:off + K] = s_node[:, None]
            off += K
        G = m_all[dst_all]                               # [128, KTOT, 128] f16
        c["G"] = np.ascontiguousarray(G.reshape(P, KTOT * D_OUT))
        c["t_s"] = t_s
        c["nv_s"] = nv_s
        c["s_s"] = s_s
    idt = np.ascontiguousarray(
        np.tile(np.eye(P, dtype=np.float16), (1, KMAX)))
    return cores, K_sched, idt


# ---------------------------------------------------------------- main ----
def kernel(x, W, a, n_vals, n_src, n_dst):
    global last_exec_times
    last_exec_times = []
    x = np.ascontiguousarray(np.asarray(x, dtype=np.float32))
    W = np.ascontiguousarray(np.asarray(W, dtype=np.float32))
    a = np.ascontiguousarray(np.asarray(a, dtype=np.float32))
    n_vals = np.ascontiguousarray(np.asarray(n_vals, dtype=np.float32))
    n_src = np.asarray(n_src, dtype=np.int64)
    n_dst = np.asarray(n_dst, dtype=np.int64)

    # ---- k1
    x_pad = np.zeros((NPAD, D_IN), dtype=np.float32)
    x_pad[:N_CELLS] = x
    a2 = np.ascontiguousarray(
        np.stack([a[:D_OUT, 0], a[D_OUT:, 0]], axis=1))  # [128, 2]
    WT = np.ascontiguousarray(W.T)
    in1 = [
        dict(xT=np.ascontiguousarray(x_pad[c * CHUNK:(c + 1) * CHUNK].T),
             Wm=W, WT=WT, a2=a2)
        for c in range(N_CORES)
    ]
    r1 = _run(_build_k1(), in1)
    m_all = np.concatenate([r1[c]["m16"] for c in range(N_CORES)], axis=0)
    s_all = np.concatenate(
        [r1[c]["st"][:, 0::2].T.reshape(-1) for c in range(N_CORES)])
    t_all = np.concatenate(
        [r1[c]["st"][:, 1::2].T.reshape(-1) for c in range(N_CORES)])

    # ---- k2
    cores, K_sched, idt = _prep_k2(n_src, n_dst, n_vals, m_all, s_all, t_all)
    in2 = [
        dict(G=c["G"], t_s=c["t_s"], nv_s=c["nv_s"], s_s=c["s_s"], idt=idt)
        for c in cores
    ]
    r2 = _run(_build_k2(K_sched), in2)

    out = np.empty((NPAD, D_OUT), dtype=np.float32)
    for c in cores:
        cc = c["c"]
        out[cc * CHUNK:(cc + 1) * CHUNK] = r2[cc]["outc"][c["rank"]]
    return np.ascontiguousarray(out[:N_CELLS])


# revision 9
# speedup vs baseline: 1.0739x; 1.0130x over previous
"""GNN message-passing (GAT-style) Trainium2 kernel.

out = (N * A) @ (x @ W) with A_ij = LeakyReLU([m_i||m_j] a) row-normalized
over the edge sparsity pattern.

Strategy (8 NeuronCores, SPMD, no collectives):
  k1: core c owns a 6272-node src slice; one fused f32 matmul per 128-node
      tile with rhs = [W | W@a_src | W@a_dst] yields m (cast to fp16) and
      the per-node scalars s = m@a_src, t = m@a_dst.
  host (indexing only, no float math): partitions edges by src slice,
      degree-sorts nodes into 49 tiles of 128, builds a k-major slot layout
      (K_i slots per tile, slot k of tile i = one edge per src node), and
      gathers the fp16 m rows of every edge's dst into one contiguous
      G tensor per core, plus slot-aligned t[dst], n_vals, s arrays.
  k2: per core: e = LeakyReLU(s+t) and w = e*n_vals in a few large vector
      ops; per tile ONE broadcast multiply builds all K diag(w_k) blocks
      from a tiled-identity constant; K PSUM-accumulated matmuls
      U = sum_k diag(w_k) @ G_k do the scatter; rs = row-sums of e; the
      PSUM->SBUF eviction applies 1/rs via the scalar engine. Final out is
      written directly (no third kernel); host just un-permutes rows.

All floating-point math runs on device; the host only shards, sorts, pads,
gathers and re-indexes.
"""

import os
import sys

for _p in ("/opt/trn_rl_repo", "/root/.axon_site/_ro/trn_rl_repo"):
    if os.path.isdir(_p) and _p not in sys.path:
        sys.path.insert(0, _p)
        break

os.environ.setdefault("MYCRO_LOCAL_CACHE", "1")

# The agent image's antenv package lacks axon_hooks; bass_utils imports it
# when tracing is requested. Install a functional shim so trace runs work
# (and degrade to no-trace when the axon .so is unavailable).
try:
    import antenv.axon_hooks  # noqa: F401
except ImportError:
    import types

    import antenv

    _hooks = types.ModuleType("antenv.axon_hooks")
    _HOOK = [None]
    _hooks.set_axon_ntff_profile_hook = lambda h: _HOOK.__setitem__(0, h)
    _hooks.get_axon_ntff_profile_hook = lambda: _HOOK[0]
    sys.modules["antenv.axon_hooks"] = _hooks
    antenv.axon_hooks = _hooks
    try:
        from trn_agent_boot.trn_boot import _ntff_profile_via_ctypes

        if os.path.exists("/opt/axon/libaxon_pjrt.so"):
            _hooks.set_axon_ntff_profile_hook(
                _ntff_profile_via_ctypes("/opt/axon/libaxon_pjrt.so")
            )
    except Exception:
        pass

import numpy as np

import concourse.bacc as bacc
import concourse.bass as bass
import concourse.mybir as mybir
import concourse.tile as tile
from concourse.bass_utils import run_bass_kernel_spmd

F32 = mybir.dt.float32
F16 = mybir.dt.float16

P = 128
N_CELLS = 50000
D_IN = 256
D_OUT = 128
NEG_SLOPE = 0.2

N_CORES = 8
NPAD = 50176          # 8 * 6272
CHUNK = 6272          # nodes per core (49 tiles)
NT1 = CHUNK // P      # 49

TRACE = bool(int(os.environ.get("GNN_TRACE", "0")))

last_exec_times = []


def _run(nc, in_maps):
    nc.compile()
    res = run_bass_kernel_spmd(
        nc, in_maps, core_ids=list(range(N_CORES)), trace=TRACE,
    )
    if res.exec_time_ns is not None:
        last_exec_times.append(res.exec_time_ns)
    return res.results


# ------------------------------------------------------------------ k1 ----
def _build_k1():
    nc = bacc.Bacc("TRN2", target_bir_lowering=False, debug=False,
                   enable_asserts=False, num_devices=N_CORES)
    xT_d = nc.dram_tensor("xT", [D_IN, CHUNK], F32, kind="ExternalInput").ap()
    W_d = nc.dram_tensor("Wm", [D_IN, D_OUT], F32, kind="ExternalInput").ap()
    WT_d = nc.dram_tensor("WT", [D_OUT, D_IN], F32, kind="ExternalInput").ap()
    a2_d = nc.dram_tensor("a2", [P, 2], F32, kind="ExternalInput").ap()
    m16_d = nc.dram_tensor("m16", [CHUNK, D_OUT], F16, kind="ExternalOutput").ap()
    st_d = nc.dram_tensor("st", [P, 2 * NT1], F32, kind="ExternalOutput").ap()

    with tile.TileContext(nc) as tc:
        with (
            tc.tile_pool(name="const", bufs=1) as cp,
            tc.tile_pool(name="xp", bufs=4) as xp,
            tc.tile_pool(name="work", bufs=4) as wp,
            tc.tile_pool(name="psum", bufs=4, space="PSUM") as pp,
        ):
            rhs0 = cp.tile([P, D_OUT + 2], F32, tag="rhs0")
            rhs1 = cp.tile([P, D_OUT + 2], F32, tag="rhs1")
            nc.sync.dma_start(out=rhs0[:, 0:D_OUT], in_=W_d[0:P, :])
            nc.sync.dma_start(out=rhs1[:, 0:D_OUT], in_=W_d[P:D_IN, :])
            WT_sb = cp.tile([P, D_IN], F32, tag="WT")
            nc.sync.dma_start(out=WT_sb[:], in_=WT_d[:])
            a2_sb = cp.tile([P, 2], F32, tag="a2")
            nc.sync.dma_start(out=a2_sb[:], in_=a2_d[:])

            # [W @ a_src | W @ a_dst] rows 0:128 and 128:256
            wa0 = pp.tile([P, 2], F32, tag="wa")
            nc.tensor.matmul(out=wa0[:], lhsT=WT_sb[:, 0:P], rhs=a2_sb[:],
                             start=True, stop=True)
            nc.vector.tensor_copy(out=rhs0[:, D_OUT:D_OUT + 2], in_=wa0[:])
            wa1 = pp.tile([P, 2], F32, tag="wa")
            nc.tensor.matmul(out=wa1[:], lhsT=WT_sb[:, P:D_IN], rhs=a2_sb[:],
                             start=True, stop=True)
            nc.vector.tensor_copy(out=rhs1[:, D_OUT:D_OUT + 2], in_=wa1[:])

            st_acc = cp.tile([P, 2 * NT1], F32, tag="stacc")
            TPC = 7                       # tiles per chunk; 49 = 7 * 7
            for ch in range(NT1 // TPC):
                c0 = ch * TPC * P
                xc0 = xp.tile([P, TPC * P], F32, tag="xc0")
                xc1 = xp.tile([P, TPC * P], F32, tag="xc1")
                nc.sync.dma_start(out=xc0[:], in_=xT_d[0:P, c0:c0 + TPC * P])
                nc.sync.dma_start(out=xc1[:], in_=xT_d[P:D_IN, c0:c0 + TPC * P])
                for j in range(TPC):
                    i = ch * TPC + j
                    mp = pp.tile([P, D_OUT + 2], F32, tag="mp")
                    nc.tensor.matmul(out=mp[:], lhsT=xc0[:, bass.ts(j, P)],
                                     rhs=rhs0[:], start=True, stop=False)
                    nc.tensor.matmul(out=mp[:], lhsT=xc1[:, bass.ts(j, P)],
                                     rhs=rhs1[:], start=False, stop=True)
                    m16t = wp.tile([P, D_OUT], F16, tag="m16t")
                    nc.vector.tensor_copy(out=m16t[:], in_=mp[:, 0:D_OUT])
                    nc.scalar.copy(out=st_acc[:, 2 * i:2 * i + 2],
                                   in_=mp[:, D_OUT:D_OUT + 2])
                    nc.sync.dma_start(out=m16_d[bass.ts(i, P), :], in_=m16t[:])
            nc.sync.dma_start(out=st_d[:], in_=st_acc[:])
    return nc


# ------------------------------------------------------------------ k2 ----
def _build_k2(K_sched):
    K_sched = [int(k) for k in K_sched]
    KTOT = sum(K_sched)
    KMAX = max(K_sched)
    nt = len(K_sched)
    nc = bacc.Bacc("TRN2", target_bir_lowering=False, debug=False,
                   enable_asserts=False, num_devices=N_CORES)
    G_d = nc.dram_tensor("G", [P, KTOT * D_OUT], F16, kind="ExternalInput").ap()
    t_d = nc.dram_tensor("t_s", [P, KTOT], F32, kind="ExternalInput").ap()
    nv_d = nc.dram_tensor("nv_s", [P, KTOT], F32, kind="ExternalInput").ap()
    s_d = nc.dram_tensor("s_s", [P, KTOT], F32, kind="ExternalInput").ap()
    idt_d = nc.dram_tensor("idt", [P, KMAX * D_OUT], F16, kind="ExternalInput").ap()
    o_d = nc.dram_tensor("outc", [CHUNK, D_OUT], F32, kind="ExternalOutput").ap()

    with tile.TileContext(nc) as tc:
        with (
            tc.tile_pool(name="const", bufs=1) as cp,
            tc.tile_pool(name="gpool", bufs=3) as gp,
            tc.tile_pool(name="dpoolv", bufs=3) as dpv,
            tc.tile_pool(name="dpoolg", bufs=3) as dpg,
            tc.tile_pool(name="opool", bufs=4) as op,
            tc.tile_pool(name="psum", bufs=4, space="PSUM") as pp,
        ):
            idt = cp.tile([P, KMAX * D_OUT], F16, tag="idt")
            nc.sync.dma_start(out=idt[:], in_=idt_d[:])
            t_sb = cp.tile([P, KTOT], F32, tag="t")
            nv_sb = cp.tile([P, KTOT], F32, tag="nv")
            s_sb = cp.tile([P, KTOT], F32, tag="s")
            nc.sync.dma_start(out=t_sb[:], in_=t_d[:])
            nc.sync.dma_start(out=nv_sb[:], in_=nv_d[:])
            nc.sync.dma_start(out=s_sb[:], in_=s_d[:])

            z_sb = cp.tile([P, KTOT], F32, tag="z")
            zp_sb = cp.tile([P, KTOT], F32, tag="zp")
            e_sb = cp.tile([P, KTOT], F32, tag="e")
            w32 = cp.tile([P, KTOT], F32, tag="w32")
            w16 = cp.tile([P, KTOT], F16, tag="w16")
            rs = cp.tile([P, nt], F32, tag="rs")
            rinv = cp.tile([P, nt], F32, tag="rinv")

            # e = LeakyReLU(t + s) = max(z,0) + NEG_SLOPE*min(z,0)
            nc.vector.tensor_add(out=z_sb[:], in0=t_sb[:], in1=s_sb[:])
            nc.vector.tensor_scalar(
                out=zp_sb[:], in0=z_sb[:], scalar1=0.0, scalar2=None,
                op0=mybir.AluOpType.max)
            nc.vector.tensor_scalar(
                out=z_sb[:], in0=z_sb[:], scalar1=0.0, scalar2=NEG_SLOPE,
                op0=mybir.AluOpType.min, op1=mybir.AluOpType.mult)
            nc.vector.tensor_add(out=e_sb[:], in0=zp_sb[:], in1=z_sb[:])
            # w = e * n_vals (f32 for scalar-engine scaling, fp16 for diags)
            nc.vector.tensor_mul(out=w32[:], in0=e_sb[:], in1=nv_sb[:])
            nc.vector.tensor_copy(out=w16[:], in_=w32[:])
            # rs_i = per-tile row sums of e; rinv = 1/rs
            off = 0
            for i in range(nt):
                K = K_sched[i]
                nc.vector.reduce_sum(out=rs[:, i:i + 1],
                                     in_=e_sb[:, off:off + K],
                                     axis=mybir.AxisListType.X)
                off += K
            nc.vector.reciprocal(out=rinv[:], in_=rs[:])

            off = 0
            for i in range(nt):
                K = K_sched[i]
                G_t = gp.tile([P, KMAX * D_OUT], F16, tag="G")
                nc.sync.dma_start(
                    out=G_t[:, 0:K * D_OUT],
                    in_=G_d[:, off * D_OUT:(off + K) * D_OUT])
                U_ps = pp.tile([P, D_OUT], F32, tag="U")
                if i % 5 == 4:
                    # scalar-engine mode: scale G rows per slot, constant
                    # identity lhsT on the PE
                    Gw = dpg.tile([P, KMAX * D_OUT], F16, tag="Gw")
                    for k in range(K):
                        nc.scalar.mul(
                            out=Gw[:, k * D_OUT:(k + 1) * D_OUT],
                            in_=G_t[:, k * D_OUT:(k + 1) * D_OUT],
                            mul=w32[:, off + k:off + k + 1])
                        nc.tensor.matmul(
                            out=U_ps[:],
                            lhsT=idt[:, 0:D_OUT],
                            rhs=Gw[:, k * D_OUT:(k + 1) * D_OUT],
                            start=(k == 0), stop=(k == K - 1))
                else:
                    diag = dpv.tile([P, KMAX * D_OUT], F16, tag="diag")
                    w_b = (w16[:, off:off + K]
                           .unsqueeze(2).broadcast_to([P, K, D_OUT]))
                    nc.vector.tensor_mul(
                        out=diag[:, 0:K * D_OUT].rearrange(
                            "p (k q) -> p k q", q=D_OUT),
                        in0=idt[:, 0:K * D_OUT].rearrange(
                            "p (k q) -> p k q", q=D_OUT),
                        in1=w_b)
                    for k in range(K):
                        nc.tensor.matmul(
                            out=U_ps[:],
                            lhsT=diag[:, k * D_OUT:(k + 1) * D_OUT],
                            rhs=G_t[:, k * D_OUT:(k + 1) * D_OUT],
                            start=(k == 0), stop=(k == K - 1))
                U_sb = op.tile([P, D_OUT], F32, tag="Usb")
                nc.scalar.mul(out=U_sb[:], in_=U_ps[:], mul=rinv[:, i:i + 1])
                nc.sync.dma_start(out=o_d[bass.ts(i, P), :], in_=U_sb[:])
                off += K
    return nc


# ------------------------------------------------------------ host prep ----
def _prep_k2(n_src, n_dst, n_vals, m_all, s_all, t_all):
    """Slot layouts per core. Pure indexing / permutation, no float math."""
    cores = []
    for c in range(N_CORES):
        sel = (n_src // CHUNK) == c
        es = n_src[sel] - c * CHUNK
        ed = n_dst[sel]
        ev = n_vals[sel]
        deg = np.bincount(es, minlength=CHUNK)
        order = np.argsort(-deg, kind="stable")       # rank -> rel id
        rank = np.empty(CHUNK, dtype=np.int64)
        rank[order] = np.arange(CHUNK)
        eo = np.argsort(rank[es], kind="stable")
        ed = ed[eo]
        ev = ev[eo]
        deg_sorted = deg[order]
        offs = np.zeros(CHUNK + 1, dtype=np.int64)
        np.cumsum(deg_sorted, out=offs[1:])
        Ks = np.maximum(
            deg_sorted.reshape(NT1, P).max(axis=1), 1).astype(np.int64)
        cores.append(dict(c=c, ed=ed, ev=ev, offs=offs, order=order,
                          rank=rank, Ks=Ks, deg_sorted=deg_sorted))
    K_sched = np.stack([c["Ks"] for c in cores]).max(axis=0)
    KTOT = int(K_sched.sum())
    KMAX = int(K_sched.max())

    for c in cores:
        cc = c["c"]
        dst_all = np.zeros((P, KTOT), dtype=np.int64)
        t_s = np.zeros((P, KTOT), dtype=np.float32)
        nv_s = np.zeros((P, KTOT), dtype=np.float32)
        s_s = np.zeros((P, KTOT), dtype=np.float32)
        off = 0
        for i in range(NT1):
            K = int(K_sched[i])
            nodes = c["order"][i * P:(i + 1) * P]        # rel ids
            s_node = s_all[cc * CHUNK + nodes]           # [128]
            d = c["deg_sorted"][i * P:(i + 1) * P]       # [128]
            base = c["offs"][i * P:(i + 1) * P]          # [128]
            sl = np.arange(K)[None, :]
            valid = sl < d[:, None]                      # [128, K]
            eidx = np.minimum(base[:, None] + sl, max(len(c["ed"]) - 1, 0))
            dst_all[:, off:off + K] = np.where(valid, c["ed"][eidx], 0)
            nv_s[:, off:off + K] = np.where(valid, c["ev"][eidx], 0.0)
            t_s[:, off:off + K] = np.where(
                valid, t_all[dst_all[:, off:off + K]], -s_node[:, None])
            s_s[:, off:off + K] = s_node[:, None]
            off += K
        G = m_all[dst_all]                               # [128, KTOT, 128] f16
        c["G"] = np.ascontiguousarray(G.reshape(P, KTOT * D_OUT))
        c["t_s"] = t_s
        c["nv_s"] = nv_s
        c["s_s"] = s_s
    idt = np.ascontiguousarray(
        np.tile(np.eye(P, dtype=np.float16), (1, KMAX)))
    return cores, K_sched, idt


# ---------------------------------------------------------------- main ----
def kernel(x, W, a, n_vals, n_src, n_dst):
    global last_exec_times
    last_exec_times = []
    x = np.ascontiguousarray(np.asarray(x, dtype=np.float32))
    W = np.ascontiguousarray(np.asarray(W, dtype=np.float32))
    a = np.ascontiguousarray(np.asarray(a, dtype=np.float32))
    n_vals = np.ascontiguousarray(np.asarray(n_vals, dtype=np.float32))
    n_src = np.asarray(n_src, dtype=np.int64)
    n_dst = np.asarray(n_dst, dtype=np.int64)

    # ---- k1
    x_pad = np.zeros((NPAD, D_IN), dtype=np.float32)
    x_pad[:N_CELLS] = x
    a2 = np.ascontiguousarray(
        np.stack([a[:D_OUT, 0], a[D_OUT:, 0]], axis=1))  # [128, 2]
    WT = np.ascontiguousarray(W.T)
    in1 = [
        dict(xT=np.ascontiguousarray(x_pad[c * CHUNK:(c + 1) * CHUNK].T),
             Wm=W, WT=WT, a2=a2)
        for c in range(N_CORES)
    ]
    r1 = _run(_build_k1(), in1)
    m_all = np.concatenate([r1[c]["m16"] for c in range(N_CORES)], axis=0)
    s_all = np.concatenate(
        [r1[c]["st"][:, 0::2].T.reshape(-1) for c in range(N_CORES)])
    t_all = np.concatenate(
        [r1[c]["st"][:, 1::2].T.reshape(-1) for c in range(N_CORES)])

    # ---- k2
    cores, K_sched, idt = _prep_k2(n_src, n_dst, n_vals, m_all, s_all, t_all)
    in2 = [
        dict(G=c["G"], t_s=c["t_s"], nv_s=c["nv_s"], s_s=c["s_s"], idt=idt)
        for c in cores
    ]
    r2 = _run(_build_k2(K_sched), in2)

    out = np.empty((NPAD, D_OUT), dtype=np.float32)
    for c in cores:
        cc = c["c"]
        out[cc * CHUNK:(cc + 1) * CHUNK] = r2[cc]["outc"][c["rank"]]
    return np.ascontiguousarray(out[:N_CELLS])


# revision 13
# speedup vs baseline: 1.1774x; 1.0964x over previous
"""GNN message-passing (GAT-style) Trainium2 kernel.

out = (N * A) @ (x @ W) with A_ij = LeakyReLU([m_i||m_j] a) row-normalized
over the edge sparsity pattern.

Strategy (8 NeuronCores, SPMD, no collectives):
  k1: core c owns a 6272-node src slice; one fused f32 matmul per 128-node
      tile with rhs = [W | W@a_src | W@a_dst] yields m (cast to fp16) and
      the per-node scalars s = m@a_src, t = m@a_dst.
  host (indexing only, no float math): partitions edges by src slice,
      degree-sorts nodes into 49 tiles of 128, builds a k-major slot layout
      (K_i slots per tile, slot k of tile i = one edge per src node), and
      gathers the fp16 m rows of every edge's dst into one contiguous
      G tensor per core, plus slot-aligned t[dst], n_vals, s arrays.
  k2: per core: e = LeakyReLU(s+t) and w = e*n_vals in a few large vector
      ops; per tile ONE broadcast multiply builds all K diag(w_k) blocks
      from a tiled-identity constant; K PSUM-accumulated matmuls
      U = sum_k diag(w_k) @ G_k do the scatter; rs = row-sums of e; the
      PSUM->SBUF eviction applies 1/rs via the scalar engine. Final out is
      written directly (no third kernel); host just un-permutes rows.

All floating-point math runs on device; the host only shards, sorts, pads,
gathers and re-indexes.
"""

import os
import sys

for _p in ("/opt/trn_rl_repo", "/root/.axon_site/_ro/trn_rl_repo"):
    if os.path.isdir(_p) and _p not in sys.path:
        sys.path.insert(0, _p)
        break

os.environ.setdefault("MYCRO_LOCAL_CACHE", "1")

# The agent image's antenv package lacks axon_hooks; bass_utils imports it
# when tracing is requested. Install a functional shim so trace runs work
# (and degrade to no-trace when the axon .so is unavailable).
try:
    import antenv.axon_hooks  # noqa: F401
except ImportError:
    import types

    import antenv

    _hooks = types.ModuleType("antenv.axon_hooks")
    _HOOK = [None]
    _hooks.set_axon_ntff_profile_hook = lambda h: _HOOK.__setitem__(0, h)
    _hooks.get_axon_ntff_profile_hook = lambda: _HOOK[0]
    sys.modules["antenv.axon_hooks"] = _hooks
    antenv.axon_hooks = _hooks
    try:
        from trn_agent_boot.trn_boot import _ntff_profile_via_ctypes

        if os.path.exists("/opt/axon/libaxon_pjrt.so"):
            _hooks.set_axon_ntff_profile_hook(
                _ntff_profile_via_ctypes("/opt/axon/libaxon_pjrt.so")
            )
    except Exception:
        pass

import numpy as np

import concourse.bacc as bacc
import concourse.bass as bass
import concourse.mybir as mybir
import concourse.tile as tile
from concourse.bass_utils import run_bass_kernel_spmd

F32 = mybir.dt.float32
F16 = mybir.dt.float16

P = 128
N_CELLS = 50000
D_IN = 256
D_OUT = 128
NEG_SLOPE = 0.2

N_CORES = 8
NPAD = 50176          # 8 * 6272
CHUNK = 6272          # nodes per core (49 tiles)
NT1 = CHUNK // P      # 49

TRACE = bool(int(os.environ.get("GNN_TRACE", "0")))

last_exec_times = []


def _run(nc, in_maps):
    nc.compile()
    res = run_bass_kernel_spmd(
        nc, in_maps, core_ids=list(range(N_CORES)), trace=TRACE,
    )
    if res.exec_time_ns is not None:
        last_exec_times.append(res.exec_time_ns)
    return res.results


# ------------------------------------------------------------------ k1 ----
def _build_k1():
    nc = bacc.Bacc("TRN2", target_bir_lowering=False, debug=False,
                   enable_asserts=False, num_devices=N_CORES)
    xT_d = nc.dram_tensor("xT", [D_IN, CHUNK], F32, kind="ExternalInput").ap()
    W_d = nc.dram_tensor("Wm", [D_IN, D_OUT], F32, kind="ExternalInput").ap()
    WT_d = nc.dram_tensor("WT", [D_OUT, D_IN], F32, kind="ExternalInput").ap()
    a2_d = nc.dram_tensor("a2", [P, 2], F32, kind="ExternalInput").ap()
    m16_d = nc.dram_tensor("m16", [CHUNK, D_OUT], F16, kind="ExternalOutput").ap()
    st_d = nc.dram_tensor("st", [P, 2 * NT1], F32, kind="ExternalOutput").ap()

    with tile.TileContext(nc) as tc:
        with (
            tc.tile_pool(name="const", bufs=1) as cp,
            tc.tile_pool(name="xp", bufs=4) as xp,
            tc.tile_pool(name="work", bufs=4) as wp,
            tc.tile_pool(name="psum", bufs=4, space="PSUM") as pp,
        ):
            rhs0 = cp.tile([P, D_OUT + 2], F32, tag="rhs0")
            rhs1 = cp.tile([P, D_OUT + 2], F32, tag="rhs1")
            nc.sync.dma_start(out=rhs0[:, 0:D_OUT], in_=W_d[0:P, :])
            nc.sync.dma_start(out=rhs1[:, 0:D_OUT], in_=W_d[P:D_IN, :])
            WT_sb = cp.tile([P, D_IN], F32, tag="WT")
            nc.sync.dma_start(out=WT_sb[:], in_=WT_d[:])
            a2_sb = cp.tile([P, 2], F32, tag="a2")
            nc.sync.dma_start(out=a2_sb[:], in_=a2_d[:])

            # [W @ a_src | W @ a_dst] rows 0:128 and 128:256
            wa0 = pp.tile([P, 2], F32, tag="wa")
            nc.tensor.matmul(out=wa0[:], lhsT=WT_sb[:, 0:P], rhs=a2_sb[:],
                             start=True, stop=True)
            nc.vector.tensor_copy(out=rhs0[:, D_OUT:D_OUT + 2], in_=wa0[:])
            wa1 = pp.tile([P, 2], F32, tag="wa")
            nc.tensor.matmul(out=wa1[:], lhsT=WT_sb[:, P:D_IN], rhs=a2_sb[:],
                             start=True, stop=True)
            nc.vector.tensor_copy(out=rhs1[:, D_OUT:D_OUT + 2], in_=wa1[:])

            st_acc = cp.tile([P, 2 * NT1], F32, tag="stacc")
            TPC = 7                       # tiles per chunk; 49 = 7 * 7
            for ch in range(NT1 // TPC):
                c0 = ch * TPC * P
                xc0 = xp.tile([P, TPC * P], F32, tag="xc0")
                xc1 = xp.tile([P, TPC * P], F32, tag="xc1")
                nc.sync.dma_start(out=xc0[:], in_=xT_d[0:P, c0:c0 + TPC * P])
                nc.sync.dma_start(out=xc1[:], in_=xT_d[P:D_IN, c0:c0 + TPC * P])
                for j in range(TPC):
                    i = ch * TPC + j
                    mp = pp.tile([P, D_OUT + 2], F32, tag="mp")
                    nc.tensor.matmul(out=mp[:], lhsT=xc0[:, bass.ts(j, P)],
                                     rhs=rhs0[:], start=True, stop=False)
                    nc.tensor.matmul(out=mp[:], lhsT=xc1[:, bass.ts(j, P)],
                                     rhs=rhs1[:], start=False, stop=True)
                    m16t = wp.tile([P, D_OUT], F16, tag="m16t")
                    nc.vector.tensor_copy(out=m16t[:], in_=mp[:, 0:D_OUT])
                    nc.scalar.copy(out=st_acc[:, 2 * i:2 * i + 2],
                                   in_=mp[:, D_OUT:D_OUT + 2])
                    nc.sync.dma_start(out=m16_d[bass.ts(i, P), :], in_=m16t[:])
            nc.sync.dma_start(out=st_d[:], in_=st_acc[:])
    return nc


# ------------------------------------------------------------------ k2 ----
def _build_k2(K_sched):
    K_sched = [int(k) for k in K_sched]
    KTOT = sum(K_sched)
    KMAX = max(K_sched)
    nt = len(K_sched)
    nc = bacc.Bacc("TRN2", target_bir_lowering=False, debug=False,
                   enable_asserts=False, num_devices=N_CORES)
    G_d = nc.dram_tensor("G", [P, KTOT * D_OUT], F16, kind="ExternalInput").ap()
    t_d = nc.dram_tensor("t_s", [P, KTOT], F32, kind="ExternalInput").ap()
    nv_d = nc.dram_tensor("nv_s", [P, KTOT], F32, kind="ExternalInput").ap()
    s_d = nc.dram_tensor("s_s", [P, KTOT], F32, kind="ExternalInput").ap()
    idt_d = nc.dram_tensor("idt", [P, KMAX * D_OUT], F16, kind="ExternalInput").ap()
    o_d = nc.dram_tensor("outc", [CHUNK, D_OUT], F32, kind="ExternalOutput").ap()

    with tile.TileContext(nc) as tc:
        with (
            tc.tile_pool(name="const", bufs=1) as cp,
            tc.tile_pool(name="gpool", bufs=4) as gp,
            tc.tile_pool(name="dpoolv", bufs=4) as dpv,
            tc.tile_pool(name="opool", bufs=6) as op,
            tc.tile_pool(name="psum", bufs=6, space="PSUM") as pp,
        ):
            idt = cp.tile([P, KMAX * D_OUT], F16, tag="idt")
            nc.sync.dma_start(out=idt[:], in_=idt_d[:])
            t_sb = cp.tile([P, KTOT], F32, tag="t")
            nv_sb = cp.tile([P, KTOT], F32, tag="nv")
            s_sb = cp.tile([P, KTOT], F32, tag="s")
            nc.sync.dma_start(out=t_sb[:], in_=t_d[:])
            nc.sync.dma_start(out=nv_sb[:], in_=nv_d[:])
            nc.sync.dma_start(out=s_sb[:], in_=s_d[:])

            z_sb = cp.tile([P, KTOT], F32, tag="z")
            zp_sb = cp.tile([P, KTOT], F32, tag="zp")
            e_sb = cp.tile([P, KTOT], F32, tag="e")
            w16 = cp.tile([P, KTOT], F16, tag="w16")
            rs = cp.tile([P, nt], F32, tag="rs")
            rinv = cp.tile([P, nt], F32, tag="rinv")

            # e = LeakyReLU(t + s) = max(z,0) + NEG_SLOPE*min(z,0)
            nc.vector.tensor_add(out=z_sb[:], in0=t_sb[:], in1=s_sb[:])
            nc.vector.tensor_scalar(
                out=zp_sb[:], in0=z_sb[:], scalar1=0.0, scalar2=None,
                op0=mybir.AluOpType.max)
            nc.vector.tensor_scalar(
                out=z_sb[:], in0=z_sb[:], scalar1=0.0, scalar2=NEG_SLOPE,
                op0=mybir.AluOpType.min, op1=mybir.AluOpType.mult)
            nc.vector.tensor_add(out=e_sb[:], in0=zp_sb[:], in1=z_sb[:])
            # w = e * n_vals (fp16 for the diag blocks)
            nc.vector.tensor_mul(out=w16[:], in0=e_sb[:], in1=nv_sb[:])
            # rs_i = per-tile row sums of e; rinv = 1/rs
            off = 0
            for i in range(nt):
                K = K_sched[i]
                nc.vector.reduce_sum(out=rs[:, i:i + 1],
                                     in_=e_sb[:, off:off + K],
                                     axis=mybir.AxisListType.X)
                off += K
            nc.vector.reciprocal(out=rinv[:], in_=rs[:])

            off = 0
            for i in range(nt):
                K = K_sched[i]
                G_t = gp.tile([P, KMAX * D_OUT], F16, tag="G")
                nc.sync.dma_start(
                    out=G_t[:, 0:K * D_OUT],
                    in_=G_d[:, off * D_OUT:(off + K) * D_OUT])
                U_ps = pp.tile([P, D_OUT], F32, tag="U")
                diag = dpv.tile([P, KMAX * D_OUT], F16, tag="diag")
                w_b = (w16[:, off:off + K]
                       .unsqueeze(2).broadcast_to([P, K, D_OUT]))
                nc.vector.tensor_mul(
                    out=diag[:, 0:K * D_OUT].rearrange(
                        "p (k q) -> p k q", q=D_OUT),
                    in0=idt[:, 0:K * D_OUT].rearrange(
                        "p (k q) -> p k q", q=D_OUT),
                    in1=w_b)
                for k in range(K):
                    nc.tensor.matmul(
                        out=U_ps[:],
                        lhsT=diag[:, k * D_OUT:(k + 1) * D_OUT],
                        rhs=G_t[:, k * D_OUT:(k + 1) * D_OUT],
                        start=(k == 0), stop=(k == K - 1))
                U_sb = op.tile([P, D_OUT], F32, tag="Usb")
                nc.scalar.mul(out=U_sb[:], in_=U_ps[:], mul=rinv[:, i:i + 1])
                nc.sync.dma_start(out=o_d[bass.ts(i, P), :], in_=U_sb[:])
                off += K
    return nc


# ------------------------------------------------------------ host prep ----
def _prep_k2(n_src, n_dst, n_vals, m_all, s_all, t_all):
    """Slot layouts per core. Pure indexing / permutation, no float math."""
    cores = []
    for c in range(N_CORES):
        sel = (n_src // CHUNK) == c
        es = n_src[sel] - c * CHUNK
        ed = n_dst[sel]
        ev = n_vals[sel]
        deg = np.bincount(es, minlength=CHUNK)
        order = np.argsort(-deg, kind="stable")       # rank -> rel id
        rank = np.empty(CHUNK, dtype=np.int64)
        rank[order] = np.arange(CHUNK)
        eo = np.argsort(rank[es], kind="stable")
        ed = ed[eo]
        ev = ev[eo]
        deg_sorted = deg[order]
        offs = np.zeros(CHUNK + 1, dtype=np.int64)
        np.cumsum(deg_sorted, out=offs[1:])
        Ks = np.maximum(
            deg_sorted.reshape(NT1, P).max(axis=1), 1).astype(np.int64)
        cores.append(dict(c=c, ed=ed, ev=ev, offs=offs, order=order,
                          rank=rank, Ks=Ks, deg_sorted=deg_sorted))
    K_sched = np.stack([c["Ks"] for c in cores]).max(axis=0)
    KTOT = int(K_sched.sum())
    KMAX = int(K_sched.max())

    for c in cores:
        cc = c["c"]
        dst_all = np.zeros((P, KTOT), dtype=np.int64)
        t_s = np.zeros((P, KTOT), dtype=np.float32)
        nv_s = np.zeros((P, KTOT), dtype=np.float32)
        s_s = np.zeros((P, KTOT), dtype=np.float32)
        off = 0
        for i in range(NT1):
            K = int(K_sched[i])
            nodes = c["order"][i * P:(i + 1) * P]        # rel ids
            s_node = s_all[cc * CHUNK + nodes]           # [128]
            d = c["deg_sorted"][i * P:(i + 1) * P]       # [128]
            base = c["offs"][i * P:(i + 1) * P]          # [128]
            sl = np.arange(K)[None, :]
            valid = sl < d[:, None]                      # [128, K]
            eidx = np.minimum(base[:, None] + sl, max(len(c["ed"]) - 1, 0))
            dst_all[:, off:off + K] = np.where(valid, c["ed"][eidx], 0)
            nv_s[:, off:off + K] = np.where(valid, c["ev"][eidx], 0.0)
            t_s[:, off:off + K] = np.where(
                valid, t_all[dst_all[:, off:off + K]], -s_node[:, None])
            s_s[:, off:off + K] = s_node[:, None]
            off += K
        G = m_all[dst_all]                               # [128, KTOT, 128] f16
        c["G"] = np.ascontiguousarray(G.reshape(P, KTOT * D_OUT))
        c["t_s"] = t_s
        c["nv_s"] = nv_s
        c["s_s"] = s_s
    idt = np.ascontiguousarray(
        np.tile(np.eye(P, dtype=np.float16), (1, KMAX)))
    return cores, K_sched, idt


# ---------------------------------------------------------------- main ----
def kernel(x, W, a, n_vals, n_src, n_dst):
    global last_exec_times
    last_exec_times = []
    x = np.ascontiguousarray(np.asarray(x, dtype=np.float32))
    W = np.ascontiguousarray(np.asarray(W, dtype=np.float32))
    a = np.ascontiguousarray(np.asarray(a, dtype=np.float32))
    n_vals = np.ascontiguousarray(np.asarray(n_vals, dtype=np.float32))
    n_src = np.asarray(n_src, dtype=np.int64)
    n_dst = np.asarray(n_dst, dtype=np.int64)

    # ---- k1
    x_pad = np.zeros((NPAD, D_IN), dtype=np.float32)
    x_pad[:N_CELLS] = x
    a2 = np.ascontiguousarray(
        np.stack([a[:D_OUT, 0], a[D_OUT:, 0]], axis=1))  # [128, 2]
    WT = np.ascontiguousarray(W.T)
    in1 = [
        dict(xT=np.ascontiguousarray(x_pad[c * CHUNK:(c + 1) * CHUNK].T),
             Wm=W, WT=WT, a2=a2)
        for c in range(N_CORES)
    ]
    r1 = _run(_build_k1(), in1)
    m_all = np.concatenate([r1[c]["m16"] for c in range(N_CORES)], axis=0)
    s_all = np.concatenate(
        [r1[c]["st"][:, 0::2].T.reshape(-1) for c in range(N_CORES)])
    t_all = np.concatenate(
        [r1[c]["st"][:, 1::2].T.reshape(-1) for c in range(N_CORES)])

    # ---- k2
    cores, K_sched, idt = _prep_k2(n_src, n_dst, n_vals, m_all, s_all, t_all)
    in2 = [
        dict(G=c["G"], t_s=c["t_s"], nv_s=c["nv_s"], s_s=c["s_s"], idt=idt)
        for c in cores
    ]
    r2 = _run(_build_k2(K_sched), in2)

    out = np.empty((NPAD, D_OUT), dtype=np.float32)
    for c in cores:
        cc = c["c"]
        out[cc * CHUNK:(cc + 1) * CHUNK] = r2[cc]["outc"][c["rank"]]
    return np.ascontiguousarray(out[:N_CELLS])


# revision 16
# speedup vs baseline: 1.1992x; 1.0185x over previous
"""GNN message-passing (GAT-style) Trainium2 kernel.

out = (N * A) @ (x @ W) with A_ij = LeakyReLU([m_i||m_j] a) row-normalized
over the edge sparsity pattern.

Strategy (8 NeuronCores, SPMD, no collectives):
  k1: core c owns a 6272-node src slice; one fused f32 matmul per 128-node
      tile with rhs = [W | W@a_src | W@a_dst] yields m (cast to fp16) and
      the per-node scalars s = m@a_src, t = m@a_dst.
  host (indexing only, no float math): partitions edges by src slice,
      degree-sorts nodes into 49 tiles of 128, builds a k-major slot layout
      (K_i slots per tile, slot k of tile i = one edge per src node), and
      gathers the fp16 m rows of every edge's dst into one contiguous
      G tensor per core, plus slot-aligned t[dst], n_vals, s arrays.
  k2: per core: e = LeakyReLU(s+t) and w = e*n_vals in a few large vector
      ops; per tile ONE broadcast multiply builds all K diag(w_k) blocks
      from a tiled-identity constant; K PSUM-accumulated matmuls
      U = sum_k diag(w_k) @ G_k do the scatter; rs = row-sums of e; the
      PSUM->SBUF eviction applies 1/rs via the scalar engine. Final out is
      written directly (no third kernel); host just un-permutes rows.

All floating-point math runs on device; the host only shards, sorts, pads,
gathers and re-indexes.
"""

import os
import sys

for _p in ("/opt/trn_rl_repo", "/root/.axon_site/_ro/trn_rl_repo"):
    if os.path.isdir(_p) and _p not in sys.path:
        sys.path.insert(0, _p)
        break

os.environ.setdefault("MYCRO_LOCAL_CACHE", "1")

# The agent image's antenv package lacks axon_hooks; bass_utils imports it
# when tracing is requested. Install a functional shim so trace runs work
# (and degrade to no-trace when the axon .so is unavailable).
try:
    import antenv.axon_hooks  # noqa: F401
except ImportError:
    import types

    import antenv

    _hooks = types.ModuleType("antenv.axon_hooks")
    _HOOK = [None]
    _hooks.set_axon_ntff_profile_hook = lambda h: _HOOK.__setitem__(0, h)
    _hooks.get_axon_ntff_profile_hook = lambda: _HOOK[0]
    sys.modules["antenv.axon_hooks"] = _hooks
    antenv.axon_hooks = _hooks
    try:
        from trn_agent_boot.trn_boot import _ntff_profile_via_ctypes

        if os.path.exists("/opt/axon/libaxon_pjrt.so"):
            _hooks.set_axon_ntff_profile_hook(
                _ntff_profile_via_ctypes("/opt/axon/libaxon_pjrt.so")
            )
    except Exception:
        pass

import numpy as np

import concourse.bacc as bacc
import concourse.bass as bass
import concourse.mybir as mybir
import concourse.tile as tile
from concourse.bass_utils import run_bass_kernel_spmd

F32 = mybir.dt.float32
F16 = mybir.dt.float16

P = 128
N_CELLS = 50000
D_IN = 256
D_OUT = 128
NEG_SLOPE = 0.2

N_CORES = 8
NPAD = 50176          # 8 * 6272
CHUNK = 6272          # nodes per core (49 tiles)
NT1 = CHUNK // P      # 49

TRACE = bool(int(os.environ.get("GNN_TRACE", "0")))

last_exec_times = []


def _run(nc, in_maps):
    nc.compile()
    res = run_bass_kernel_spmd(
        nc, in_maps, core_ids=list(range(N_CORES)), trace=TRACE,
    )
    if res.exec_time_ns is not None:
        last_exec_times.append(res.exec_time_ns)
    return res.results


# ------------------------------------------------------------------ k1 ----
def _build_k1():
    nc = bacc.Bacc("TRN2", target_bir_lowering=False, debug=False,
                   enable_asserts=False, num_devices=N_CORES)
    xT_d = nc.dram_tensor("xT", [D_IN, CHUNK], F32, kind="ExternalInput").ap()
    W_d = nc.dram_tensor("Wm", [D_IN, D_OUT], F32, kind="ExternalInput").ap()
    WT_d = nc.dram_tensor("WT", [D_OUT, D_IN], F32, kind="ExternalInput").ap()
    a2_d = nc.dram_tensor("a2", [P, 2], F32, kind="ExternalInput").ap()
    m16_d = nc.dram_tensor("m16", [CHUNK, D_OUT], F16, kind="ExternalOutput").ap()
    st_d = nc.dram_tensor("st", [P, 2 * NT1], F32, kind="ExternalOutput").ap()

    with tile.TileContext(nc) as tc:
        with (
            tc.tile_pool(name="const", bufs=1) as cp,
            tc.tile_pool(name="xp", bufs=6) as xp,
            tc.tile_pool(name="work", bufs=6) as wp,
            tc.tile_pool(name="psum", bufs=4, space="PSUM") as pp,
        ):
            rhs0 = cp.tile([P, D_OUT + 2], F32, tag="rhs0")
            rhs1 = cp.tile([P, D_OUT + 2], F32, tag="rhs1")
            nc.sync.dma_start(out=rhs0[:, 0:D_OUT], in_=W_d[0:P, :])
            nc.sync.dma_start(out=rhs1[:, 0:D_OUT], in_=W_d[P:D_IN, :])
            WT_sb = cp.tile([P, D_IN], F32, tag="WT")
            nc.sync.dma_start(out=WT_sb[:], in_=WT_d[:])
            a2_sb = cp.tile([P, 2], F32, tag="a2")
            nc.sync.dma_start(out=a2_sb[:], in_=a2_d[:])

            # [W @ a_src | W @ a_dst] rows 0:128 and 128:256
            wa0 = pp.tile([P, 2], F32, tag="wa")
            nc.tensor.matmul(out=wa0[:], lhsT=WT_sb[:, 0:P], rhs=a2_sb[:],
                             start=True, stop=True)
            nc.vector.tensor_copy(out=rhs0[:, D_OUT:D_OUT + 2], in_=wa0[:])
            wa1 = pp.tile([P, 2], F32, tag="wa")
            nc.tensor.matmul(out=wa1[:], lhsT=WT_sb[:, P:D_IN], rhs=a2_sb[:],
                             start=True, stop=True)
            nc.vector.tensor_copy(out=rhs1[:, D_OUT:D_OUT + 2], in_=wa1[:])

            st_acc = cp.tile([P, 2 * NT1], F32, tag="stacc")
            TPC = 7                       # tiles per chunk; 49 = 7 * 7
            for ch in range(NT1 // TPC):
                c0 = ch * TPC * P
                xc0 = xp.tile([P, TPC * P], F32, tag="xc0")
                xc1 = xp.tile([P, TPC * P], F32, tag="xc1")
                nc.sync.dma_start(out=xc0[:], in_=xT_d[0:P, c0:c0 + TPC * P])
                nc.sync.dma_start(out=xc1[:], in_=xT_d[P:D_IN, c0:c0 + TPC * P])
                for j in range(TPC):
                    i = ch * TPC + j
                    mp = pp.tile([P, D_OUT + 2], F32, tag="mp")
                    nc.tensor.matmul(out=mp[:], lhsT=xc0[:, bass.ts(j, P)],
                                     rhs=rhs0[:], start=True, stop=False)
                    nc.tensor.matmul(out=mp[:], lhsT=xc1[:, bass.ts(j, P)],
                                     rhs=rhs1[:], start=False, stop=True)
                    m16t = wp.tile([P, D_OUT], F16, tag="m16t")
                    nc.vector.tensor_copy(out=m16t[:], in_=mp[:, 0:D_OUT])
                    nc.scalar.copy(out=st_acc[:, 2 * i:2 * i + 2],
                                   in_=mp[:, D_OUT:D_OUT + 2])
                    nc.sync.dma_start(out=m16_d[bass.ts(i, P), :], in_=m16t[:])
            nc.sync.dma_start(out=st_d[:], in_=st_acc[:])
    return nc


# ------------------------------------------------------------------ k2 ----
def _build_k2(K_sched):
    K_sched = [int(k) for k in K_sched]
    KTOT = sum(K_sched)
    KMAX = max(K_sched)
    nt = len(K_sched)
    nc = bacc.Bacc("TRN2", target_bir_lowering=False, debug=False,
                   enable_asserts=False, num_devices=N_CORES)
    G_d = nc.dram_tensor("G", [P, KTOT * D_OUT], F16, kind="ExternalInput").ap()
    t_d = nc.dram_tensor("t_s", [P, KTOT], F32, kind="ExternalInput").ap()
    nv_d = nc.dram_tensor("nv_s", [P, KTOT], F32, kind="ExternalInput").ap()
    s_d = nc.dram_tensor("s_s", [P, KTOT], F32, kind="ExternalInput").ap()
    idt_d = nc.dram_tensor("idt", [P, KMAX * D_OUT], F16, kind="ExternalInput").ap()
    o_d = nc.dram_tensor("outc", [CHUNK, D_OUT], F32, kind="ExternalOutput").ap()

    with tile.TileContext(nc) as tc:
        with (
            tc.tile_pool(name="const", bufs=1) as cp,
            tc.tile_pool(name="gpool", bufs=4) as gp,
            tc.tile_pool(name="dpoolv", bufs=4) as dpv,
            tc.tile_pool(name="opool", bufs=6) as op,
            tc.tile_pool(name="psum", bufs=6, space="PSUM") as pp,
        ):
            t_sb = cp.tile([P, KTOT], F32, tag="t")
            nv_sb = cp.tile([P, KTOT], F32, tag="nv")
            s_sb = cp.tile([P, KTOT], F32, tag="s")
            nc.sync.dma_start(out=t_sb[:], in_=t_d[:])
            nc.sync.dma_start(out=nv_sb[:], in_=nv_d[:])
            nc.sync.dma_start(out=s_sb[:], in_=s_d[:])
            idt = cp.tile([P, KMAX * D_OUT], F16, tag="idt")
            nc.sync.dma_start(out=idt[:], in_=idt_d[:])

            z_sb = cp.tile([P, KTOT], F32, tag="z")
            zp_sb = cp.tile([P, KTOT], F32, tag="zp")
            e_sb = cp.tile([P, KTOT], F32, tag="e")
            w16 = cp.tile([P, KTOT], F16, tag="w16")
            rs = cp.tile([P, nt], F32, tag="rs")
            rinv = cp.tile([P, nt], F32, tag="rinv")

            # e = LeakyReLU(t + s) = max(z,0) + NEG_SLOPE*min(z,0); w = e*nv.
            # Prefix slots (first two tiles) first so diag #0 starts early.
            K01 = K_sched[0] + (K_sched[1] if nt > 1 else 0)
            for lo, hi in ((0, K01), (K01, KTOT)):
                sl = slice(lo, hi)
                nc.vector.tensor_add(out=z_sb[:, sl], in0=t_sb[:, sl],
                                     in1=s_sb[:, sl])
                nc.vector.tensor_scalar(
                    out=zp_sb[:, sl], in0=z_sb[:, sl], scalar1=0.0,
                    scalar2=None, op0=mybir.AluOpType.max)
                nc.vector.tensor_scalar(
                    out=z_sb[:, sl], in0=z_sb[:, sl], scalar1=0.0,
                    scalar2=NEG_SLOPE, op0=mybir.AluOpType.min,
                    op1=mybir.AluOpType.mult)
                nc.vector.tensor_add(out=e_sb[:, sl], in0=zp_sb[:, sl],
                                     in1=z_sb[:, sl])
                nc.vector.tensor_mul(out=w16[:, sl], in0=e_sb[:, sl],
                                     in1=nv_sb[:, sl])
            # rs_i = per-tile row sums of e; rinv = 1/rs
            off = 0
            for i in range(nt):
                K = K_sched[i]
                nc.vector.reduce_sum(out=rs[:, i:i + 1],
                                     in_=e_sb[:, off:off + K],
                                     axis=mybir.AxisListType.X)
                off += K
            nc.vector.reciprocal(out=rinv[:], in_=rs[:])

            off = 0
            for i in range(nt):
                K = K_sched[i]
                G_t = gp.tile([P, KMAX * D_OUT], F16, tag="G")
                nc.sync.dma_start(
                    out=G_t[:, 0:K * D_OUT],
                    in_=G_d[:, off * D_OUT:(off + K) * D_OUT])
                U_ps = pp.tile([P, D_OUT], F32, tag="U")
                diag = dpv.tile([P, KMAX * D_OUT], F16, tag="diag")
                w_b = (w16[:, off:off + K]
                       .unsqueeze(2).broadcast_to([P, K, D_OUT]))
                nc.vector.tensor_mul(
                    out=diag[:, 0:K * D_OUT].rearrange(
                        "p (k q) -> p k q", q=D_OUT),
                    in0=idt[:, 0:K * D_OUT].rearrange(
                        "p (k q) -> p k q", q=D_OUT),
                    in1=w_b)
                for k in range(K):
                    nc.tensor.matmul(
                        out=U_ps[:],
                        lhsT=diag[:, k * D_OUT:(k + 1) * D_OUT],
                        rhs=G_t[:, k * D_OUT:(k + 1) * D_OUT],
                        start=(k == 0), stop=(k == K - 1))
                U_sb = op.tile([P, D_OUT], F32, tag="Usb")
                nc.scalar.mul(out=U_sb[:], in_=U_ps[:], mul=rinv[:, i:i + 1])
                nc.sync.dma_start(out=o_d[bass.ts(i, P), :], in_=U_sb[:])
                off += K
    return nc


# ------------------------------------------------------------ host prep ----
def _prep_k2(n_src, n_dst, n_vals, m_all, s_all, t_all):
    """Slot layouts per core. Pure indexing / permutation, no float math."""
    cores = []
    for c in range(N_CORES):
        sel = (n_src // CHUNK) == c
        es = n_src[sel] - c * CHUNK
        ed = n_dst[sel]
        ev = n_vals[sel]
        deg = np.bincount(es, minlength=CHUNK)
        order = np.argsort(-deg, kind="stable")       # rank -> rel id
        rank = np.empty(CHUNK, dtype=np.int64)
        rank[order] = np.arange(CHUNK)
        eo = np.argsort(rank[es], kind="stable")
        ed = ed[eo]
        ev = ev[eo]
        deg_sorted = deg[order]
        offs = np.zeros(CHUNK + 1, dtype=np.int64)
        np.cumsum(deg_sorted, out=offs[1:])
        Ks = np.maximum(
            deg_sorted.reshape(NT1, P).max(axis=1), 1).astype(np.int64)
        cores.append(dict(c=c, ed=ed, ev=ev, offs=offs, order=order,
                          rank=rank, Ks=Ks, deg_sorted=deg_sorted))
    K_sched = np.stack([c["Ks"] for c in cores]).max(axis=0)
    KTOT = int(K_sched.sum())
    KMAX = int(K_sched.max())

    for c in cores:
        cc = c["c"]
        dst_all = np.zeros((P, KTOT), dtype=np.int64)
        t_s = np.zeros((P, KTOT), dtype=np.float32)
        nv_s = np.zeros((P, KTOT), dtype=np.float32)
        s_s = np.zeros((P, KTOT), dtype=np.float32)
        off = 0
        for i in range(NT1):
            K = int(K_sched[i])
            nodes = c["order"][i * P:(i + 1) * P]        # rel ids
            s_node = s_all[cc * CHUNK + nodes]           # [128]
            d = c["deg_sorted"][i * P:(i + 1) * P]       # [128]
            base = c["offs"][i * P:(i + 1) * P]          # [128]
            sl = np.arange(K)[None, :]
            valid = sl < d[:, None]                      # [128, K]
            eidx = np.minimum(base[:, None] + sl, max(len(c["ed"]) - 1, 0))
            dst_all[:, off:off + K] = np.where(valid, c["ed"][eidx], 0)
            nv_s[:, off:off + K] = np.where(valid, c["ev"][eidx], 0.0)
            t_s[:, off:off + K] = np.where(
                valid, t_all[dst_all[:, off:off + K]], -s_node[:, None])
            s_s[:, off:off + K] = s_node[:, None]
            off += K
        G = m_all[dst_all]                               # [128, KTOT, 128] f16
        c["G"] = np.ascontiguousarray(G.reshape(P, KTOT * D_OUT))
        c["t_s"] = t_s
        c["nv_s"] = nv_s
        c["s_s"] = s_s
    idt = np.ascontiguousarray(
        np.tile(np.eye(P, dtype=np.float16), (1, KMAX)))
    return cores, K_sched, idt


# ---------------------------------------------------------------- main ----
def kernel(x, W, a, n_vals, n_src, n_dst):
    global last_exec_times
    last_exec_times = []
    x = np.ascontiguousarray(np.asarray(x, dtype=np.float32))
    W = np.ascontiguousarray(np.asarray(W, dtype=np.float32))
    a = np.ascontiguousarray(np.asarray(a, dtype=np.float32))
    n_vals = np.ascontiguousarray(np.asarray(n_vals, dtype=np.float32))
    n_src = np.asarray(n_src, dtype=np.int64)
    n_dst = np.asarray(n_dst, dtype=np.int64)

    # ---- k1
    x_pad = np.zeros((NPAD, D_IN), dtype=np.float32)
    x_pad[:N_CELLS] = x
    a2 = np.ascontiguousarray(
        np.stack([a[:D_OUT, 0], a[D_OUT:, 0]], axis=1))  # [128, 2]
    WT = np.ascontiguousarray(W.T)
    in1 = [
        dict(xT=np.ascontiguousarray(x_pad[c * CHUNK:(c + 1) * CHUNK].T),
             Wm=W, WT=WT, a2=a2)
        for c in range(N_CORES)
    ]
    r1 = _run(_build_k1(), in1)
    m_all = np.concatenate([r1[c]["m16"] for c in range(N_CORES)], axis=0)
    s_all = np.concatenate(
        [r1[c]["st"][:, 0::2].T.reshape(-1) for c in range(N_CORES)])
    t_all = np.concatenate(
        [r1[c]["st"][:, 1::2].T.reshape(-1) for c in range(N_CORES)])

    # ---- k2
    cores, K_sched, idt = _prep_k2(n_src, n_dst, n_vals, m_all, s_all, t_all)
    in2 = [
        dict(G=c["G"], t_s=c["t_s"], nv_s=c["nv_s"], s_s=c["s_s"], idt=idt)
        for c in cores
    ]
    r2 = _run(_build_k2(K_sched), in2)

    out = np.empty((NPAD, D_OUT), dtype=np.float32)
    for c in cores:
        cc = c["c"]
        out[cc * CHUNK:(cc + 1) * CHUNK] = r2[cc]["outc"][c["rank"]]
    return np.ascontiguousarray(out[:N_CELLS])
